# revision 1
# baseline (speedup 1.0000x reference)
"""Trainium2 Bass kernel for nn_CustomLoss_30743375905383.

loss = sum_i[ (p0-(1-t))^2 + (p1-t)^2 + 2*[wrong] ] / N
  where wrong = (t==0 ? p0<p1 : p1<p0)

Data-parallel over 8 NeuronCores: core c handles N/8 consecutive rows.
Per core, with x the interleaved pred block [p0 p1 p0 p1 ...],
d = p1-p0 and u = t*d, the partial sum decomposes into four streaming
reductions (free-dim accumulate on ScalarE/VectorE, no matmuls):

  A = sum x^2         ScalarE Square + accumulate
  B = sum p0          ScalarE Copy on even lanes + accumulate
  U = sum t*d         VectorE scalar_tensor_tensor bypass/mult + accumulate
  G = count(2u < d)   VectorE scalar_tensor_tensor mult/is_lt + accumulate

  partial = A + R - 2B - 2U + 2G      (R = rows per core)
  loss = sum(partials) / N

Each core streams its 24 MiB shard once from HBM (memory-bound); the
host combines the 8 tiny accumulator tensors in float64.
"""

import sys

if "/opt/trn_rl_repo" not in sys.path:
    sys.path.insert(0, "/opt/trn_rl_repo")

import numpy as np
import concourse.bass as bass
import concourse.mybir as mybir
import concourse.tile as tile
from concourse.bass_utils import run_bass_kernel_spmd

F32 = mybir.dt.float32
I32 = mybir.dt.int32
AF = mybir.ActivationFunctionType
ALU = mybir.AluOpType

P = 128                          # SBUF partitions
N_TOTAL = 16777216
N_CORES = 8
R = N_TOTAL // N_CORES           # rows (pairs) per core = 2097152
W = 2 * R // P                   # f32 pred elems per partition = 32768
W2 = R // P                      # int32 targets per partition = 16384

TILE_C = 4096                    # pred elems per partition per tile
IO_BUFS = 4
MID_BUFS = 2


def _split_excess_waits(nc, max_waits=1):
    """This walrus build's CoreV3 codegen caps sem-wait commands per
    instruction; split excess waits onto preceding same-engine no-ops.
    Engines run their stream in order and the waits are monotonic
    sem-ge conditions, so sequential chunked waits are equivalent."""
    counter = [0]

    def fresh_name(base):
        counter[0] += 1
        return f"{base}-wsplit{counter[0]}"

    for fn in nc.m.functions:
        for bb in fn.blocks:
            out = []
            changed = False
            for inst in bb.instructions:
                si = inst.sync_info
                waits = list(si.on_wait) if si is not None else []
                if len(waits) > max_waits:
                    changed = True
                    head, tail = waits[:-max_waits], waits[-max_waits:]
                    for i in range(0, len(head), max_waits):
                        out.append(mybir.InstNoOp(
                            name=fresh_name(inst.name),
                            sync_info=mybir.SyncInfo(
                                on_wait=head[i:i + max_waits], on_update=[]),
                            bass_nofuse=True,
                            engine=inst.engine,
                        ))
                    inst.sync_info = mybir.SyncInfo(
                        on_wait=tail, on_update=list(si.on_update))
                out.append(inst)
            if changed:
                bb.instructions = out


def _build(C=TILE_C, io_bufs=IO_BUFS, mid_bufs=MID_BUFS):
    NT = W // C
    F = C // 2
    nc = bass.Bass(trn_type="TRN2", target_bir_lowering=False, debug=False)
    pred = nc.dram_tensor("pred", [P, W], F32, kind="ExternalInput").ap()
    targ = nc.dram_tensor("targ", [P, W2], I32, kind="ExternalInput").ap()
    out_acc = nc.dram_tensor("out_acc", [P, 4 * NT], F32,
                             kind="ExternalOutput").ap()

    with tile.TileContext(nc) as tc:
        with tc.tile_pool(name="io", bufs=io_bufs) as io_pool, \
             tc.tile_pool(name="mid", bufs=mid_bufs) as mid_pool, \
             tc.tile_pool(name="accs", bufs=1) as acc_pool:
            accA = acc_pool.tile([P, NT], F32)
            accB = acc_pool.tile([P, NT], F32)
            accU = acc_pool.tile([P, NT], F32)
            accG = acc_pool.tile([P, NT], F32)
            for i in range(NT):
                X = io_pool.tile([P, C], F32, tag="X")
                T = io_pool.tile([P, F], I32, tag="T")
                # equal ~1MB chunks (X halved, T whole) keep the HWDGE
                # queues balanced and concurrently busy
                h = C // 2
                nc.sync.dma_start(X[:, :h], pred[:, i * C:i * C + h])
                nc.sync.dma_start(X[:, h:], pred[:, i * C + h:(i + 1) * C])
                nc.sync.dma_start(T[:], targ[:, i * F:(i + 1) * F])

                x2 = mid_pool.tile([P, C], F32, tag="x2")
                p0c = mid_pool.tile([P, F], F32, tag="p0c")
                dT = mid_pool.tile([P, F], F32, tag="dT")
                uT = mid_pool.tile([P, F], F32, tag="uT")
                gc = mid_pool.tile([P, F], F32, tag="gc")

                # A: sum of squares of all pred elems
                nc.scalar.activation(x2[:], X[:], AF.Square,
                                     accum_out=accA[:, i:i + 1])
                # B: sum of p0 (even lanes)
                nc.scalar.activation(p0c[:], X[:, ::2], AF.Copy,
                                     accum_out=accB[:, i:i + 1])
                # d = p1 - p0
                nc.vector.tensor_tensor(dT[:], X[:, 1::2], X[:, ::2],
                                        ALU.subtract)
                # u = t*d (int32 t converted on the fly) ; U = sum u
                nc.vector.scalar_tensor_tensor(
                    uT[:], T[:], 0.0, dT[:], ALU.bypass, ALU.mult,
                    accum_out=accU[:, i:i + 1])
                # wrong = (2u < d) ; G = count
                nc.vector.scalar_tensor_tensor(
                    gc[:], uT[:], 2.0, dT[:], ALU.mult, ALU.is_lt,
                    accum_out=accG[:, i:i + 1])

            nc.sync.dma_start(out_acc[:, 0 * NT:1 * NT], accA[:])
            nc.sync.dma_start(out_acc[:, 1 * NT:2 * NT], accB[:])
            nc.sync.dma_start(out_acc[:, 2 * NT:3 * NT], accU[:])
            nc.sync.dma_start(out_acc[:, 3 * NT:4 * NT], accG[:])

    _split_excess_waits(nc, max_waits=1)
    return nc, NT


_CACHE = {}


def _get_program():
    if "prog" not in _CACHE:
        _CACHE["prog"] = _build()
    return _CACHE["prog"]


def kernel(pred, target):
    pred = np.asarray(pred)
    target = np.asarray(target)
    assert pred.shape == (N_TOTAL, 2) and pred.dtype == np.float32
    if target.dtype != np.int32:
        # jax without x64 hands us int32; accept int64 too (values are 0/1)
        target = target.astype(np.int32)

    nc, NT = _get_program()
    in_maps = []
    for c in range(N_CORES):
        ps = pred[c * R:(c + 1) * R].reshape(P, W)
        ts = target[c * R:(c + 1) * R].reshape(P, W2)
        in_maps.append({"pred": np.ascontiguousarray(ps),
                        "targ": np.ascontiguousarray(ts)})

    res = run_bass_kernel_spmd(nc, in_maps, list(range(N_CORES)))

    total = 0.0
    for r in res.results:
        acc = np.asarray(r["out_acc"]).astype(np.float64)
        A = acc[:, 0 * NT:1 * NT].sum()
        B = acc[:, 1 * NT:2 * NT].sum()
        U = acc[:, 2 * NT:3 * NT].sum()
        G = acc[:, 3 * NT:4 * NT].sum()
        total += A + R - 2.0 * B - 2.0 * U + 2.0 * G
    return np.float32(total / N_TOTAL)



# revision 4
# speedup vs baseline: 1.1432x; 1.1432x over previous
"""Trainium2 Bass kernel for nn_CustomLoss_30743375905383.

loss = sum_i[ (p0-(1-t))^2 + (p1-t)^2 + 2*[wrong] ] / N
  where wrong = (t==0 ? p0<p1 : p1<p0)

Math used here (s = 1-2t in {+1,-1}, e = p0-p1, q = s*e):
  sq_i   = (p0-1)^2 + p1^2 + 2*t*e          (exact identity)
  sum(t*e) = (E - Q)/2  with E = sum(e), Q = sum(q)
  wrong  = q < 0 ; ties (q==0, only possible after quantization)
           are counted via alternating is_lt / is_le per tile, which is
           unbiased (a tie's true penalty is 0 or 2 with equal odds).
  loss_sum = SA + SB + E - Q + 2*G
    SA = sum (p0-1)^2   (ScalarE Square with bias=-1, free affine)
    SB = sum p1^2       (split ScalarE Square / VectorE mult to balance)
    G  = count(q < 0)   (VectorE tensor_scalar is_lt/is_le, 4x mode)

Data-parallel over 8 NeuronCores: core c handles N/8 consecutive rows.

Bandwidth strategy (memory-bound problem, rel-err budget 2e-2):
  pred is quantized host-side to fp8e4 (rel err ~5e-4 on the loss) and
  target recoded to s in {+1,-1} fp8e4. HBM traffic is 6 MiB/core
  (vs 24 MiB for the f32 baseline). SWDGE (gpsimd) DMAs upcast
  fp8 -> bf16 on the fly so all DVE ops run in 16-bit 2x/4x perf modes.
"""

import sys

if "/opt/trn_rl_repo" not in sys.path:
    sys.path.insert(0, "/opt/trn_rl_repo")

import numpy as np
import ml_dtypes
import concourse.bass as bass
import concourse.mybir as mybir
import concourse.tile as tile
from concourse.bass_utils import run_bass_kernel_spmd

F32 = mybir.dt.float32
BF16 = mybir.dt.bfloat16
F8 = mybir.dt.float8e4
AF = mybir.ActivationFunctionType
ALU = mybir.AluOpType

P = 128                          # SBUF partitions
N_TOTAL = 16777216
N_CORES = 8
R = N_TOTAL // N_CORES           # pairs per core = 2097152
W = R // P                       # pairs per partition = 16384

TILE_C = 4096                    # pair-columns per tile
NT = W // TILE_C                 # 4 tiles
# columns of p1^2 done on ScalarE; the rest go to VectorE (engine balance)
SQ1_SCALAR_COLS = TILE_C - 1280

IO_BUFS = 3
MID_BUFS = 2


def _split_excess_waits(nc, max_waits=1):
    """This walrus build's CoreV3 codegen caps sem-wait commands per
    instruction; split excess waits onto preceding same-engine no-ops.
    Engines run their stream in order and the waits are monotonic
    sem-ge conditions, so sequential chunked waits are equivalent."""
    counter = [0]

    def fresh_name(base):
        counter[0] += 1
        return f"{base}-wsplit{counter[0]}"

    for fn in nc.m.functions:
        for bb in fn.blocks:
            out = []
            changed = False
            for inst in bb.instructions:
                si = inst.sync_info
                waits = list(si.on_wait) if si is not None else []
                if len(waits) > max_waits:
                    changed = True
                    head, tail = waits[:-max_waits], waits[-max_waits:]
                    for i in range(0, len(head), max_waits):
                        out.append(mybir.InstNoOp(
                            name=fresh_name(inst.name),
                            sync_info=mybir.SyncInfo(
                                on_wait=head[i:i + max_waits], on_update=[]),
                            bass_nofuse=True,
                            engine=inst.engine,
                        ))
                    inst.sync_info = mybir.SyncInfo(
                        on_wait=tail, on_update=list(si.on_update))
                out.append(inst)
            if changed:
                bb.instructions = out


def _build(C=TILE_C, io_bufs=IO_BUFS, mid_bufs=MID_BUFS):
    nt = W // C
    cs = SQ1_SCALAR_COLS         # scalar part of p1^2
    nc = bass.Bass(trn_type="TRN2", target_bir_lowering=False, debug=False)
    p0 = nc.dram_tensor("p0", [P, W], F8, kind="ExternalInput").ap()
    p1 = nc.dram_tensor("p1", [P, W], F8, kind="ExternalInput").ap()
    sg = nc.dram_tensor("sg", [P, W], F8, kind="ExternalInput").ap()
    # acc layout: [SA | SBs | SBv | E | Q | G] each nt wide
    out_acc = nc.dram_tensor("out_acc", [P, 6 * nt], F32,
                             kind="ExternalOutput").ap()

    with tile.TileContext(nc) as tc:
        with tc.tile_pool(name="io", bufs=io_bufs) as io_pool, \
             tc.tile_pool(name="mid", bufs=mid_bufs) as mid_pool, \
             tc.tile_pool(name="accs", bufs=1) as acc_pool:
            acc = acc_pool.tile([P, 6 * nt], F32)
            for i in range(nt):
                P0b = io_pool.tile([P, C], BF16, tag="P0b")
                P1b = io_pool.tile([P, C], BF16, tag="P1b")
                Sb = io_pool.tile([P, C], BF16, tag="Sb")
                # SWDGE cast-DMA: fp8 in HBM -> bf16 in SBUF
                nc.gpsimd.dma_start(P0b[:], p0[:, i * C:(i + 1) * C])
                nc.gpsimd.dma_start(P1b[:], p1[:, i * C:(i + 1) * C])
                nc.gpsimd.dma_start(Sb[:], sg[:, i * C:(i + 1) * C])

                scrA = mid_pool.tile([P, C], BF16, tag="scrA")
                scrB = mid_pool.tile([P, cs], BF16, tag="scrB")
                scrV = mid_pool.tile([P, C - cs], BF16, tag="scrV")
                eT = mid_pool.tile([P, C], BF16, tag="eT")
                qT = mid_pool.tile([P, C], BF16, tag="qT")
                gT = mid_pool.tile([P, C], BF16, tag="gT")

                # SA += sum (1 - p0)^2           [ScalarE, free affine]
                nc.scalar.activation(scrA[:], P0b[:], AF.Square,
                                     bias=1.0, scale=-1.0,
                                     accum_out=acc[:, i:i + 1])
                # SB (scalar part) += sum p1^2   [ScalarE]
                nc.scalar.activation(scrB[:], P1b[:, :cs], AF.Square,
                                     accum_out=acc[:, nt + i:nt + i + 1])
                # SB (vector part) += sum p1^2   [VectorE, 2x bf16]
                nc.vector.scalar_tensor_tensor(
                    scrV[:], P1b[:, cs:], 0.0, P1b[:, cs:],
                    ALU.bypass, ALU.mult,
                    accum_out=acc[:, 2 * nt + i:2 * nt + i + 1])
                # e = p0 - p1 ; E += sum e       [VectorE, 2x bf16]
                nc.vector.scalar_tensor_tensor(
                    eT[:], P0b[:], 0.0, P1b[:],
                    ALU.bypass, ALU.subtract,
                    accum_out=acc[:, 3 * nt + i:3 * nt + i + 1])
                # q = s * e ; Q += sum q         [VectorE, 2x bf16]
                nc.vector.scalar_tensor_tensor(
                    qT[:], Sb[:], 0.0, eT[:],
                    ALU.bypass, ALU.mult,
                    accum_out=acc[:, 4 * nt + i:4 * nt + i + 1])
                # G += count(q < 0) (alt <= on odd tiles for unbiased ties)
                cmp = ALU.is_lt if i % 2 == 0 else ALU.is_le
                nc.vector.tensor_scalar(
                    gT[:], qT[:], 0.0, None, cmp, ALU.add,
                    accum_out=acc[:, 5 * nt + i:5 * nt + i + 1])

            nc.sync.dma_start(out_acc[:], acc[:])

    _split_excess_waits(nc, max_waits=1)
    return nc, nt


_CACHE = {}


def _get_program():
    if "prog" not in _CACHE:
        _CACHE["prog"] = _build()
    return _CACHE["prog"]


def kernel(pred, target):
    pred = np.asarray(pred)
    target = np.asarray(target)
    assert pred.shape == (N_TOTAL, 2) and pred.dtype == np.float32
    f8 = ml_dtypes.float8_e4m3
    p0_8 = np.ascontiguousarray(pred[:, 0]).astype(f8)
    p1_8 = np.ascontiguousarray(pred[:, 1]).astype(f8)
    s_8 = (1 - 2 * target.astype(np.int32)).astype(np.float32).astype(f8)

    nc, nt = _get_program()
    in_maps = []
    for c in range(N_CORES):
        sl = slice(c * R, (c + 1) * R)
        in_maps.append({
            "p0": p0_8[sl].reshape(P, W),
            "p1": p1_8[sl].reshape(P, W),
            "sg": s_8[sl].reshape(P, W),
        })

    res = run_bass_kernel_spmd(nc, in_maps, list(range(N_CORES)))

    total = 0.0
    for r in res.results:
        acc = np.asarray(r["out_acc"]).astype(np.float64)
        SA = acc[:, 0 * nt:1 * nt].sum()
        SBs = acc[:, 1 * nt:2 * nt].sum()
        SBv = acc[:, 2 * nt:3 * nt].sum()
        E = acc[:, 3 * nt:4 * nt].sum()
        Q = acc[:, 4 * nt:5 * nt].sum()
        G = acc[:, 5 * nt:6 * nt].sum()
        total += SA + SBs + SBv + E - Q + 2.0 * G
    return np.float32(total / N_TOTAL)


# revision 5
# speedup vs baseline: 1.4169x; 1.2393x over previous
"""Trainium2 Bass kernel for nn_CustomLoss_30743375905383.

loss = sum_i[ (p0-(1-t))^2 + (p1-t)^2 + 2*[wrong] ] / N
  where wrong = (t==0 ? p0<p1 : p1<p0)

Math (s = 1-2t in {+1,-1}, e = p0-p1, q = s*e):
  sq_i   = (1-p0)^2 + p1^2 + 2*t*e          (exact identity)
  sum(t*e) = (E - Q)/2,  E = sum(e), Q = sum(q)
  wrong  = q < 0 ; quantization ties (q==0) are counted via alternating
           is_lt / is_le per tile (a tie's true penalty is 0 or 2 with
           equal odds, so alternating strict/non-strict is unbiased).
  loss_sum = SA + SB + E - Q + 2*G
    SA = sum (1-p0)^2,  SB = sum p1^2,  G = count(wrong)

Data-parallel over 8 NeuronCores: core c handles N/8 consecutive rows.

Engine assignment (all per-core roofs ~25-30us, memory-bound):
  HWDGE DMA : p0, p1 planes as bf16 (8 MiB/core HBM; 2e-2 rel-err budget
              makes bf16 safe, measured ~1e-4).
  SWDGE DMA : s as fp8 in HBM (2 MiB), cast fp8->bf16 in-flight so DVE
              ops stay in 16-bit 2x mode.
  ScalarE   : Square(1-p0) via free affine (scale=-1,bias=1) and most of
              sum p1^2, using the activation's native accumulator.
  VectorE   : pure elementwise bf16 ops (no accum_out - the DVE
              accumulate uop only runs 1x; plain TT=2x, TS=4x).
  TensorE   : all remaining reductions as ones-stationary matmuls
              accumulating into [1,512] PSUM banks (PE is otherwise
              idle; matmul accum groups span all tiles).
"""

import sys

if "/opt/trn_rl_repo" not in sys.path:
    sys.path.insert(0, "/opt/trn_rl_repo")

import numpy as np
import ml_dtypes
import concourse.bass as bass
import concourse.mybir as mybir
import concourse.tile as tile
from concourse.bass_utils import run_bass_kernel_spmd

F32 = mybir.dt.float32
BF16 = mybir.dt.bfloat16
F8 = mybir.dt.float8e4
AF = mybir.ActivationFunctionType
ALU = mybir.AluOpType

P = 128                          # SBUF partitions
N_TOTAL = 16777216
N_CORES = 8
R = N_TOTAL // N_CORES           # pairs per core = 2097152
W = R // P                       # pairs per partition = 16384

TILE_C = 4096                    # pair-columns per tile
NT = W // TILE_C                 # 4 tiles
VCOLS = 1024                     # columns of p1^2 done on VectorE
CS = TILE_C - VCOLS              # columns of p1^2 done on ScalarE
MM_C = 512                       # matmul chunk (one PSUM bank row)

IO_BUFS = 3
MID_BUFS = 2


def _split_excess_waits(nc, max_waits=1):
    """This walrus build's CoreV3 codegen caps sem-wait commands per
    instruction; split excess waits onto preceding same-engine no-ops.
    Engines run their stream in order and the waits are monotonic
    sem-ge conditions, so sequential chunked waits are equivalent."""
    counter = [0]

    def fresh_name(base):
        counter[0] += 1
        return f"{base}-wsplit{counter[0]}"

    for fn in nc.m.functions:
        for bb in fn.blocks:
            out = []
            changed = False
            for inst in bb.instructions:
                si = inst.sync_info
                waits = list(si.on_wait) if si is not None else []
                if len(waits) > max_waits:
                    changed = True
                    head, tail = waits[:-max_waits], waits[-max_waits:]
                    for i in range(0, len(head), max_waits):
                        out.append(mybir.InstNoOp(
                            name=fresh_name(inst.name),
                            sync_info=mybir.SyncInfo(
                                on_wait=head[i:i + max_waits], on_update=[]),
                            bass_nofuse=True,
                            engine=inst.engine,
                        ))
                    inst.sync_info = mybir.SyncInfo(
                        on_wait=tail, on_update=list(si.on_update))
                out.append(inst)
            if changed:
                bb.instructions = out


def _build(C=TILE_C, io_bufs=IO_BUFS, mid_bufs=MID_BUFS):
    nt = W // C
    nc = bass.Bass(trn_type="TRN2", target_bir_lowering=False, debug=False)
    p0 = nc.dram_tensor("p0", [P, W], BF16, kind="ExternalInput").ap()
    p1 = nc.dram_tensor("p1", [P, W], BF16, kind="ExternalInput").ap()
    sg = nc.dram_tensor("sg", [P, W], F8, kind="ExternalInput").ap()
    # [ SA | SBs ] per tile from the activation accumulators
    out_acc = nc.dram_tensor("out_acc", [P, 2 * nt], F32,
                             kind="ExternalOutput").ap()
    # [ E | Q | G | SBv ] from the PE psum accumulators (partition 0)
    out_red = nc.dram_tensor("out_red", [1, 4 * MM_C], F32,
                             kind="ExternalOutput").ap()

    ones = nc.const_aps.tensor(1.0, (P, 1), BF16)

    with tile.TileContext(nc) as tc:
        with tc.tile_pool(name="io", bufs=io_bufs) as io_pool, \
             tc.tile_pool(name="mid", bufs=mid_bufs) as mid_pool, \
             tc.tile_pool(name="accs", bufs=1) as acc_pool, \
             tc.psum_pool(name="red", bufs=1) as red_pool:
            acc = acc_pool.tile([P, 2 * nt], F32)
            red = red_pool.tile([1, 4 * MM_C], F32)
            red_sb = acc_pool.tile([1, 4 * MM_C], F32)
            for i in range(nt):
                P0b = io_pool.tile([P, C], BF16, tag="P0b")
                P1b = io_pool.tile([P, C], BF16, tag="P1b")
                Sb = io_pool.tile([P, C], BF16, tag="Sb")
                nc.sync.dma_start(P0b[:], p0[:, i * C:(i + 1) * C])
                nc.sync.dma_start(P1b[:], p1[:, i * C:(i + 1) * C])
                # SWDGE cast-DMA: fp8 in HBM -> bf16 in SBUF
                nc.gpsimd.dma_start(Sb[:], sg[:, i * C:(i + 1) * C])

                scrA = mid_pool.tile([P, C], BF16, tag="scrA")
                scrV = mid_pool.tile([P, VCOLS], BF16, tag="scrV")
                eT = mid_pool.tile([P, C], BF16, tag="eT")
                qT = mid_pool.tile([P, C], BF16, tag="qT")
                gT = mid_pool.tile([P, C], BF16, tag="gT")

                # SA += sum (1 - p0)^2           [ScalarE, native accum]
                nc.scalar.activation(scrA[:], P0b[:], AF.Square,
                                     bias=1.0, scale=-1.0,
                                     accum_out=acc[:, i:i + 1])
                # SB (scalar part) += sum p1^2   [ScalarE, native accum]
                nc.scalar.activation(scrA[:, :CS], P1b[:, :CS], AF.Square,
                                     accum_out=acc[:, nt + i:nt + i + 1])
                # SB (vector part): p1^2 elementwise [VectorE 2x bf16]
                nc.vector.tensor_tensor(scrV[:], P1b[:, CS:], P1b[:, CS:],
                                        ALU.mult)
                # e = p0 - p1                    [VectorE 2x bf16]
                nc.vector.tensor_tensor(eT[:], P0b[:], P1b[:], ALU.subtract)
                # q = s * e                      [VectorE 2x bf16]
                nc.vector.tensor_tensor(qT[:], Sb[:], eT[:], ALU.mult)
                # g = [q < 0] (alt <= on odd tiles for unbiased ties)
                cmp = ALU.is_lt if i % 2 == 0 else ALU.is_le
                nc.vector.tensor_scalar(gT[:], qT[:], 0.0, None, cmp)

                # PE reductions: ones^T @ chunk accumulates column sums
                # into [1, MM_C] psum rows across all tiles.
                for red_idx, src in ((0, eT), (1, qT), (2, gT)):
                    for c in range(C // MM_C):
                        nc.tensor.matmul(
                            red[:, red_idx * MM_C:(red_idx + 1) * MM_C],
                            ones,
                            src[:, c * MM_C:(c + 1) * MM_C],
                            start=(i == 0 and c == 0),
                            stop=(i == nt - 1 and c == C // MM_C - 1),
                        )
                for c in range(VCOLS // MM_C):
                    nc.tensor.matmul(
                        red[:, 3 * MM_C:4 * MM_C],
                        ones,
                        scrV[:, c * MM_C:(c + 1) * MM_C],
                        start=(i == 0 and c == 0),
                        stop=(i == nt - 1 and c == VCOLS // MM_C - 1),
                    )

            # PSUM has no DMA route: bounce through SBUF.
            nc.vector.tensor_copy(red_sb[:], red[:])
            nc.sync.dma_start(out_acc[:], acc[:])
            nc.sync.dma_start(out_red[:], red_sb[:])

    _split_excess_waits(nc, max_waits=1)
    return nc, nt


_CACHE = {}


def _get_program():
    if "prog" not in _CACHE:
        _CACHE["prog"] = _build()
    return _CACHE["prog"]


def kernel(pred, target):
    pred = np.asarray(pred)
    target = np.asarray(target)
    assert pred.shape == (N_TOTAL, 2) and pred.dtype == np.float32
    bf16 = ml_dtypes.bfloat16
    f8 = ml_dtypes.float8_e4m3
    p0_h = np.ascontiguousarray(pred[:, 0]).astype(bf16)
    p1_h = np.ascontiguousarray(pred[:, 1]).astype(bf16)
    s_h = (1 - 2 * target.astype(np.int32)).astype(np.float32).astype(f8)

    nc, nt = _get_program()
    in_maps = []
    for c in range(N_CORES):
        sl = slice(c * R, (c + 1) * R)
        in_maps.append({
            "p0": p0_h[sl].reshape(P, W),
            "p1": p1_h[sl].reshape(P, W),
            "sg": s_h[sl].reshape(P, W),
        })

    res = run_bass_kernel_spmd(nc, in_maps, list(range(N_CORES)))

    total = 0.0
    for r in res.results:
        acc = np.asarray(r["out_acc"]).astype(np.float64)
        red = np.asarray(r["out_red"]).astype(np.float64).reshape(-1)
        SA = acc[:, 0 * nt:1 * nt].sum()
        SBs = acc[:, 1 * nt:2 * nt].sum()
        E = red[0 * MM_C:1 * MM_C].sum()
        Q = red[1 * MM_C:2 * MM_C].sum()
        G = red[2 * MM_C:3 * MM_C].sum()
        SBv = red[3 * MM_C:4 * MM_C].sum()
        total += SA + SBs + SBv + E - Q + 2.0 * G
    return np.float32(total / N_TOTAL)
